# revision 18
# baseline (speedup 1.0000x reference)
"""Trainium2 Bass kernel for nn_CapsuleLayer_9852654977072.

The reference module collapses mathematically: the routing loop's coupling
logits `b` stay zero (faithfully-reproduced bug in the original torch code),
so routing coefficients are a fixed spatial map r(h,w) = 1/(8*cnt(h,w)) where
cnt is the 5x5 box-count inside the image. The whole module is therefore:

    p = conv2d(u as [N,64,H,W], Wd as [128,64,5,5], pad=2) * s(h,w)
    v = squash_z1(p)   # groups of 16 channels
    out[n,t1,z1,h,w] = v

Device strategy (8 cores, SPMD): shard (batch n in 0..3) x (row-half in 0..1).
Each core computes all 128 output channels for 64 rows of one image.

Conv: inputs shipped as XA/XC [128, 68, 132] bf16 whose partition halves hold
u shifted by (+0row,+1row) and (+2row+0col,+2row+1col) respectively, columns
padded by 2. Per 4-row block, 13 PSUM-accumulated bf16 matmuls (N=512) cover
all 25 taps: 10 XA row-pairs + 2 XC col-pairs + 1 K=64 single. bf16 operands
enable fast-weight-load on the PE and halve the input/output DMA traffic
(conv accumulation stays fp32 in PSUM; end-to-end rel err ~4e-3).

Squash: per block, a block-diag matmul reduces p^2 over z1 into an 8-row
slice of a shared [128, 512] PSUM tile (partition = 8*block_in_group + t1,
8 blocks per group). The factor F = y/((1+y)sqrt(y_raw+eps)), y = s^2*y_raw
is then computed ONCE per group on all 128 partitions (instead of per-block
on 8 partitions, which left 94% of the vector lanes idle and made GpSimd the
bottleneck at ~9us per tiny op). Per block, an expand matmul (K=8) fans F
back out to 128 channels and v = p * F is written out in bf16.
"""

import numpy as np
import ml_dtypes

BF16 = ml_dtypes.bfloat16

T0, Z0, T1, Z1, KK, PAD = 4, 16, 8, 16, 5, 2
N, H, W_SP = 4, 128, 128
CIN, COUT = T0 * Z0, T1 * Z1  # 64, 128
N_CORES = 8
ROWS = 64          # output rows per core
XROWS = 68         # input rows incl. halo
XCOLS = 132        # 128 + 2*PAD
BLK = 4            # output rows per block
N_BLKS = ROWS // BLK   # 16
GRP = 4            # blocks per factor group (PSUM slots at partition 0/32/64/96)
N_GRPS = (N_BLKS + GRP - 1) // GRP  # 4
SLOT = 32          # partition stride between blocks within a group

# conv matmul j -> (source, row_off, col_off); weights match in _weight_tiles
# odd dx taps read from XB (XA shifted left one column) so every matmul's
# moving operand starts at an even bf16 column (4-byte aligned streams are
# ~1.5x faster than odd-offset ones on the PE rhs path)
_MM_SLICES = (
    [(('XB', dy + 2, dx + 1) if dx in (-1, 1) else ('XA', dy + 2, dx + 2))
     for dy in (-2, 0) for dx in (-2, -1, 0, 1, 2)]
    + [('XC', 2, 0), ('XC', 2, 2), ('XC', 2, 4)]
)

_CACHE = {}


def _weight_tiles(W):
    Wd = W.transpose(1, 0, 2, 3, 4).reshape(COUT, CIN, KK, KK)
    wl = np.zeros((128, 13, 128), np.float32)  # [k, j, m]
    j = 0
    for dy in (-2, 0):
        for dx in (-2, -1, 0, 1, 2):
            wl[0:64, j, :] = Wd[:, :, dy + 2, dx + 2].T
            wl[64:128, j, :] = Wd[:, :, dy + 3, dx + 2].T
            j += 1
    for dx0 in (-2, 0):
        wl[0:64, j, :] = Wd[:, :, 4, dx0 + 2].T
        wl[64:128, j, :] = Wd[:, :, 4, dx0 + 3].T
        j += 1
    wl[0:64, j, :] = Wd[:, :, 4, 4].T  # single tap (2,2) on lo partitions
    return wl


def _inputs_core(x, half):
    """x: [64, H, W] one image channel-major. Returns XA, XC [128, 68, 132]."""
    base = half * 64 - 2
    XA = np.zeros((128, XROWS, XCOLS), np.float32)
    XC = np.zeros((128, XROWS, XCOLS), np.float32)

    def fill(dst, roff, c0, c1):
        lo, hi = max(0, -(base + roff)), min(XROWS, H - base - roff)
        dst[:, lo:hi, c0:c1] = x[:, base + roff + lo:base + roff + hi, :]

    fill(XA[0:64], 0, 2, 130)
    fill(XA[64:128], 1, 2, 130)
    fill(XC[0:64], 2, 2, 130)
    fill(XC[64:128], 2, 1, 129)
    XB = np.zeros((128, XROWS, XCOLS), np.float32)
    XB[:, :, 0:XCOLS - 1] = XA[:, :, 1:XCOLS]
    return XA, XB, XC


def _s2x_map(half):
    """[128, N_GRPS*GRP*BLK*128] f32: partition p=8j+t1 holds s^2 for block
    g*GRP+j laid out as [g, BLK*128]."""
    idx = np.arange(H)
    cnt = (np.minimum(idx + 2, H - 1) - np.maximum(idx - 2, 0) + 1).astype(np.float64)
    s = 1.0 / (8.0 * cnt[:, None] * cnt[None, :])  # [H, W]
    s = s[half * 64:(half + 1) * 64, :]
    s2 = (s * s).astype(np.float32).reshape(N_BLKS, BLK * 128)
    out = np.zeros((128, N_GRPS, BLK * 128), np.float32)
    for blk in range(N_BLKS):
        g, j = blk // GRP, blk % GRP
        for t1 in range(T1):
            out[SLOT * j + t1, g] = s2[blk]
    return np.ascontiguousarray(out.reshape(128, N_GRPS * BLK * 128))


def _block_diag():
    bd = np.zeros((128, 8), np.float32)
    bd[np.arange(128), np.arange(128) // 16] = 1.0
    return bd


def build_nc():
    import concourse.bass as bass
    import concourse.bacc as bacc
    import concourse.mybir as mybir
    import concourse.tile as tile

    f32 = mybir.dt.float32
    bf16 = mybir.dt.bfloat16
    AF = mybir.ActivationFunctionType

    nc = bacc.Bacc(None, target_bir_lowering=False)
    xa_d = nc.dram_tensor("xa", [128, XROWS * XCOLS], bf16, kind="ExternalInput")
    xb_d = nc.dram_tensor("xb", [128, XROWS * XCOLS], bf16, kind="ExternalInput")
    xc_d = nc.dram_tensor("xc", [128, XROWS * XCOLS], bf16, kind="ExternalInput")
    wl_d = nc.dram_tensor("wl", [128, 13 * 128], bf16, kind="ExternalInput")
    bd_d = nc.dram_tensor("bd", [128, 8], bf16, kind="ExternalInput")
    ex_d = nc.dram_tensor("ex", [8, 128], bf16, kind="ExternalInput")
    s2x_d = nc.dram_tensor("s2x", [128, N_GRPS * BLK * 128], f32,
                           kind="ExternalInput")
    out_d = nc.dram_tensor("out", [128, ROWS * 128], bf16, kind="ExternalOutput")

    with tile.TileContext(nc) as tc:
        with (
            tc.tile_pool(name="consts", bufs=1) as consts,
            tc.tile_pool(name="pall", bufs=N_BLKS) as pall,
            tc.tile_pool(name="work", bufs=4) as work,
            tc.tile_pool(name="fac", bufs=2) as fac,
            tc.tile_pool(name="pp", bufs=3, space="PSUM") as pp,
            tc.tile_pool(name="pf", bufs=3, space="PSUM") as pf,
            tc.tile_pool(name="py", bufs=2, space="PSUM") as py,
        ):
            # ordering matters: the first conv block needs xa/xc rows 0..16
            # and the weights; everything else can land during compute.
            xa = consts.tile([128, XROWS, XCOLS], bf16)
            xb = consts.tile([128, XROWS, XCOLS], bf16)
            xc = consts.tile([128, XROWS, XCOLS], bf16)
            wl = consts.tile([128, 13, 128], bf16)
            xa_src = xa_d.ap().rearrange("p (r c) -> p r c", c=XCOLS)
            xb_src = xb_d.ap().rearrange("p (r c) -> p r c", c=XCOLS)
            xc_src = xc_d.ap().rearrange("p (r c) -> p r c", c=XCOLS)
            CH0 = 17
            nc.sync.dma_start(out=xa[:, 0:CH0, :], in_=xa_src[:, 0:CH0, :])
            nc.scalar.dma_start(
                out=wl, in_=wl_d.ap().rearrange("p (j m) -> p j m", m=128))
            nc.sync.dma_start(out=xb[:, 0:CH0, :], in_=xb_src[:, 0:CH0, :])
            nc.sync.dma_start(out=xc[:, 0:CH0, :], in_=xc_src[:, 0:CH0, :])
            for c0 in range(CH0, XROWS, 17):
                ce = min(c0 + 17, XROWS)
                nc.sync.dma_start(
                    out=xa[:, c0:ce, :], in_=xa_src[:, c0:ce, :])
                nc.sync.dma_start(
                    out=xb[:, c0:ce, :], in_=xb_src[:, c0:ce, :])
                nc.sync.dma_start(
                    out=xc[:, c0:ce, :], in_=xc_src[:, c0:ce, :])
            bd = consts.tile([128, 8], bf16)
            nc.sync.dma_start(out=bd, in_=bd_d.ap())
            # expand matrix replicated at base partitions 0/32/64 (matmul
            # requires lhsT and rhs to share a base partition)
            ex = consts.tile([SLOT * (GRP - 1) + 8, 128], bf16)
            for j in range(GRP):
                nc.sync.dma_start(out=ex[SLOT * j:SLOT * j + 8, :],
                                  in_=ex_d.ap())
            s2x = consts.tile([128, N_GRPS, BLK * 128], f32)
            nc.sync.dma_start(
                out=s2x,
                in_=s2x_d.ap().rearrange("p (g e) -> p g e", e=BLK * 128))
            eps_t = consts.tile([128, 1], f32)
            nc.vector.memset(eps_t[:], 1e-9)

            out_v = out_d.ap().rearrange("p (r c) -> p r c", c=128)

            # PE warm-up: ~40 throwaway matmuls on scratch SBUF while the
            # input DMA lands. Keeps the PE continuously busy from ~+1us so
            # the HAM throttle reaches its full-clock state before the first
            # real conv matmul (cold-start matmuls otherwise run ~1.5x slow).
            scratch = consts.tile([128, 512], bf16)
            nc.vector.memset(scratch[:], 0.0)
            warm_ps = pf.tile([128, BLK, 128], f32, name="warm_ps", tag="fe")
            for _ in range(10):
                nc.tensor.matmul(warm_ps[:], scratch[:, 0:128], scratch[:],
                                 start=True, stop=True)

            y_all = [None] * N_GRPS
            F_g = [None] * N_GRPS
            p_sb = [None] * N_BLKS

            def stageA(blk):
                """conv 13 MMs -> p_ps; psq (ACT); p_sb copy (DVE); bd MM into
                the group's shared y PSUM tile at partitions [8j, 8j+8)."""
                g, j = blk // GRP, blk % GRP
                r0 = blk * BLK
                p_ps = pp.tile([128, BLK, 128], f32)
                for m, (src, roff, coff) in enumerate(_MM_SLICES):
                    xsrc = {'XA': xa, 'XB': xb, 'XC': xc}[src]
                    if m == 12:  # K=64 single on lo partitions
                        lhsT = wl[0:64, m, :]
                        rhs = xsrc[0:64, r0 + roff:r0 + roff + BLK,
                                   coff:coff + 128]
                    else:
                        lhsT = wl[:, m, :]
                        rhs = xsrc[:, r0 + roff:r0 + roff + BLK, coff:coff + 128]
                    nc.tensor.matmul(p_ps[:], lhsT, rhs,
                                     start=(m == 0), stop=(m == 12))
                psq = work.tile([128, BLK, 128], bf16, tag="psq")
                nc.scalar.activation(psq[:], p_ps[:], AF.Square)
                pb = pall.tile([128, BLK, 128], bf16, tag="p_sb")
                nc.scalar.activation(pb[:], p_ps[:], AF.Copy, bias=0.0)
                p_sb[blk] = pb
                if j == 0:
                    y_all[g] = py.tile([128, BLK, 128], f32, tag="y_all",
                                       name=f"y_all_{g}")
                nc.tensor.matmul(y_all[g][SLOT * j:SLOT * j + 8, :, :], bd[:],
                                 psq[:], start=True, stop=True,
                                 tile_position=(0, SLOT * j))

            def stageB(g):
                """factor on all 128 partitions: F = y/((1+y)*sqrt(y_raw+eps)),
                y = s^2 * y_raw, for 8 blocks at once."""
                y_ps = y_all[g]
                a_t = fac.tile([128, BLK, 128], f32, tag="a")
                nc.scalar.activation(a_t[:], y_ps[:], AF.Sqrt, bias=eps_t[:])
                y_t = fac.tile([128, BLK, 128], f32, tag="y")
                nc.vector.tensor_mul(
                    y_t[:], y_ps[:],
                    s2x[:, g, :].rearrange("p (r c) -> p r c", c=128))
                y1_t = fac.tile([128, BLK, 128], f32, tag="y1")
                nc.vector.tensor_scalar_add(y1_t[:], y_t[:], 1.0)
                b_t = fac.tile([128, BLK, 128], f32, tag="b")
                nc.vector.tensor_mul(b_t[:], a_t[:], y1_t[:])
                r_t = fac.tile([128, BLK, 128], f32, tag="r")
                nc.vector.reciprocal_approx_fast(r_t[:], b_t[:])
                Ft = fac.tile([128, BLK, 128], bf16, tag="F")
                nc.vector.tensor_mul(Ft[:], y_t[:], r_t[:])
                F_g[g] = Ft

            def stageC(blk):
                """expand F (K=8 MM) and v = p * F -> bf16 -> HBM."""
                g, j = blk // GRP, blk % GRP
                r0 = blk * BLK
                fe_ps = pf.tile([128, BLK, 128], f32, tag="fe")
                nc.tensor.matmul(fe_ps[:], ex[SLOT * j:SLOT * j + 8, :],
                                 F_g[g][SLOT * j:SLOT * j + 8, :, :],
                                 start=True, stop=True,
                                 tile_position=(SLOT * j, 0))
                v_t = work.tile([128, BLK, 128], bf16, tag="v")
                nc.vector.tensor_mul(v_t[:], p_sb[blk][:], fe_ps[:])
                nc.sync.dma_start(out=out_v[:, r0:r0 + BLK, :], in_=v_t[:])

            for blk in range(N_BLKS):
                stageA(blk)
                if blk % GRP == GRP - 1 or blk == N_BLKS - 1:
                    stageB(blk // GRP)
                if blk >= GRP:
                    stageC(blk - GRP)
            for blk in range(N_BLKS - GRP, N_BLKS):
                stageC(blk)

    nc.compile()
    return nc


def _prep_in_maps(u, W):
    x = u.reshape(N, CIN, H, W_SP)
    wl = _weight_tiles(W).reshape(128, 13 * 128).astype(BF16)
    bd = _block_diag().astype(BF16)
    ex = np.ascontiguousarray(_block_diag().T).astype(BF16)
    in_maps = []
    for core in range(N_CORES):
        n, half = core // 2, core % 2
        XA, XB, XC = _inputs_core(x[n], half)
        in_maps.append({
            "xa": XA.reshape(128, XROWS * XCOLS).astype(BF16),
            "xb": XB.reshape(128, XROWS * XCOLS).astype(BF16),
            "xc": XC.reshape(128, XROWS * XCOLS).astype(BF16),
            "wl": wl,
            "bd": bd,
            "ex": ex,
            "s2x": _s2x_map(half),
        })
    return in_maps


def run(u, W, trace=False):
    """Returns (out [N,T1,Z1,H,W] f32, BassKernelResults)."""
    from concourse.bass_utils import run_bass_kernel_spmd

    if "nc" not in _CACHE:
        _CACHE["nc"] = build_nc()
    nc = _CACHE["nc"]
    in_maps = _prep_in_maps(np.asarray(u, np.float32), np.asarray(W, np.float32))
    res = run_bass_kernel_spmd(nc, in_maps, list(range(N_CORES)), trace=trace)
    out = np.empty((N, T1, Z1, H, W_SP), np.float32)
    for core in range(N_CORES):
        n, half = core // 2, core % 2
        o = np.asarray(res.results[core]["out"]).astype(np.float32)
        o = o.reshape(T1, Z1, ROWS, 128)
        out[n, :, :, half * 64:(half + 1) * 64, :] = o
    return out, res


def kernel(u, W):
    out, _ = run(u, W, trace=False)
    return out


# revision 19
# speedup vs baseline: 1.0198x; 1.0198x over previous
"""Trainium2 Bass kernel for nn_CapsuleLayer_9852654977072.

The reference module collapses mathematically: the routing loop's coupling
logits `b` stay zero (faithfully-reproduced bug in the original torch code),
so routing coefficients are a fixed spatial map r(h,w) = 1/(8*cnt(h,w)) where
cnt is the 5x5 box-count inside the image. The whole module is therefore:

    p = conv2d(u as [N,64,H,W], Wd as [128,64,5,5], pad=2) * s(h,w)
    v = squash_z1(p)   # groups of 16 channels
    out[n,t1,z1,h,w] = v

Device strategy (8 cores, SPMD): shard (batch n in 0..3) x (row-half in 0..1).
Each core computes all 128 output channels for 64 rows of one image.

Conv: inputs shipped as XA/XC [128, 68, 132] bf16 whose partition halves hold
u shifted by (+0row,+1row) and (+2row+0col,+2row+1col) respectively, columns
padded by 2. Per 4-row block, 13 PSUM-accumulated bf16 matmuls (N=512) cover
all 25 taps: 10 XA row-pairs + 2 XC col-pairs + 1 K=64 single. bf16 operands
enable fast-weight-load on the PE and halve the input/output DMA traffic
(conv accumulation stays fp32 in PSUM; end-to-end rel err ~4e-3).

Squash: per block, a block-diag matmul reduces p^2 over z1 into an 8-row
slice of a shared [128, 512] PSUM tile (partition = 8*block_in_group + t1,
8 blocks per group). The factor F = y/((1+y)sqrt(y_raw+eps)), y = s^2*y_raw
is then computed ONCE per group on all 128 partitions (instead of per-block
on 8 partitions, which left 94% of the vector lanes idle and made GpSimd the
bottleneck at ~9us per tiny op). Per block, an expand matmul (K=8) fans F
back out to 128 channels and v = p * F is written out in bf16.
"""

import numpy as np
import ml_dtypes

BF16 = ml_dtypes.bfloat16

T0, Z0, T1, Z1, KK, PAD = 4, 16, 8, 16, 5, 2
N, H, W_SP = 4, 128, 128
CIN, COUT = T0 * Z0, T1 * Z1  # 64, 128
N_CORES = 8
ROWS = 64          # output rows per core
XROWS = 68         # input rows incl. halo
XCOLS = 132        # 128 + 2*PAD
BLK = 4            # output rows per block
N_BLKS = ROWS // BLK   # 16
GRP = 3            # blocks per factor group (PSUM slots at partition 0/32/64)
N_GRPS = (N_BLKS + GRP - 1) // GRP  # 6
SLOT = 32          # partition stride between blocks within a group

# conv matmul j -> (source, row_off, col_off); weights match in _weight_tiles
# odd dx taps read from XB (XA shifted left one column) so every matmul's
# moving operand starts at an even bf16 column (4-byte aligned streams are
# ~1.5x faster than odd-offset ones on the PE rhs path)
_MM_SLICES = (
    [(('XB', dy + 2, dx + 1) if dx in (-1, 1) else ('XA', dy + 2, dx + 2))
     for dy in (-2, 0) for dx in (-2, -1, 0, 1, 2)]
    + [('XC', 2, 0), ('XC', 2, 2), ('XC', 2, 4)]
)

_CACHE = {}


def _weight_tiles(W):
    Wd = W.transpose(1, 0, 2, 3, 4).reshape(COUT, CIN, KK, KK)
    wl = np.zeros((128, 13, 128), np.float32)  # [k, j, m]
    j = 0
    for dy in (-2, 0):
        for dx in (-2, -1, 0, 1, 2):
            wl[0:64, j, :] = Wd[:, :, dy + 2, dx + 2].T
            wl[64:128, j, :] = Wd[:, :, dy + 3, dx + 2].T
            j += 1
    for dx0 in (-2, 0):
        wl[0:64, j, :] = Wd[:, :, 4, dx0 + 2].T
        wl[64:128, j, :] = Wd[:, :, 4, dx0 + 3].T
        j += 1
    wl[0:64, j, :] = Wd[:, :, 4, 4].T  # single tap (2,2) on lo partitions
    return wl


def _inputs_core(x, half):
    """x: [64, H, W] one image channel-major. Returns XA, XC [128, 68, 132]."""
    base = half * 64 - 2
    XA = np.zeros((128, XROWS, XCOLS), np.float32)
    XC = np.zeros((128, XROWS, XCOLS), np.float32)

    def fill(dst, roff, c0, c1):
        lo, hi = max(0, -(base + roff)), min(XROWS, H - base - roff)
        dst[:, lo:hi, c0:c1] = x[:, base + roff + lo:base + roff + hi, :]

    fill(XA[0:64], 0, 2, 130)
    fill(XA[64:128], 1, 2, 130)
    fill(XC[0:64], 2, 2, 130)
    fill(XC[64:128], 2, 1, 129)
    XB = np.zeros((128, XROWS, XCOLS), np.float32)
    XB[:, :, 0:XCOLS - 1] = XA[:, :, 1:XCOLS]
    return XA, XB, XC


def _s2x_map(half):
    """[128, N_GRPS*GRP*BLK*128] f32: partition p=8j+t1 holds s^2 for block
    g*GRP+j laid out as [g, BLK*128]."""
    idx = np.arange(H)
    cnt = (np.minimum(idx + 2, H - 1) - np.maximum(idx - 2, 0) + 1).astype(np.float64)
    s = 1.0 / (8.0 * cnt[:, None] * cnt[None, :])  # [H, W]
    s = s[half * 64:(half + 1) * 64, :]
    s2 = (s * s).astype(np.float32).reshape(N_BLKS, BLK * 128)
    out = np.zeros((128, N_GRPS, BLK * 128), np.float32)
    for blk in range(N_BLKS):
        g, j = blk // GRP, blk % GRP
        for t1 in range(T1):
            out[SLOT * j + t1, g] = s2[blk]
    return np.ascontiguousarray(out.reshape(128, N_GRPS * BLK * 128))


def _block_diag():
    bd = np.zeros((128, 8), np.float32)
    bd[np.arange(128), np.arange(128) // 16] = 1.0
    return bd


def build_nc():
    import concourse.bass as bass
    import concourse.bacc as bacc
    import concourse.mybir as mybir
    import concourse.tile as tile

    f32 = mybir.dt.float32
    bf16 = mybir.dt.bfloat16
    AF = mybir.ActivationFunctionType

    nc = bacc.Bacc(None, target_bir_lowering=False)
    xa_d = nc.dram_tensor("xa", [128, XROWS * XCOLS], bf16, kind="ExternalInput")
    xb_d = nc.dram_tensor("xb", [128, XROWS * XCOLS], bf16, kind="ExternalInput")
    xc_d = nc.dram_tensor("xc", [128, XROWS * XCOLS], bf16, kind="ExternalInput")
    wl_d = nc.dram_tensor("wl", [128, 13 * 128], bf16, kind="ExternalInput")
    bd_d = nc.dram_tensor("bd", [128, 8], bf16, kind="ExternalInput")
    ex_d = nc.dram_tensor("ex", [8, 128], bf16, kind="ExternalInput")
    s2x_d = nc.dram_tensor("s2x", [128, N_GRPS * BLK * 128], f32,
                           kind="ExternalInput")
    out_d = nc.dram_tensor("out", [128, ROWS * 128], bf16, kind="ExternalOutput")

    with tile.TileContext(nc) as tc:
        with (
            tc.tile_pool(name="consts", bufs=1) as consts,
            tc.tile_pool(name="pall", bufs=N_BLKS) as pall,
            tc.tile_pool(name="work", bufs=4) as work,
            tc.tile_pool(name="fac", bufs=2) as fac,
            tc.tile_pool(name="pp", bufs=3, space="PSUM") as pp,
            tc.tile_pool(name="pf", bufs=3, space="PSUM") as pf,
            tc.tile_pool(name="py", bufs=2, space="PSUM") as py,
        ):
            # ordering matters: the first conv block needs xa/xc rows 0..16
            # and the weights; everything else can land during compute.
            xa = consts.tile([128, XROWS, XCOLS], bf16)
            xb = consts.tile([128, XROWS, XCOLS], bf16)
            xc = consts.tile([128, XROWS, XCOLS], bf16)
            wl = consts.tile([128, 13, 128], bf16)
            xa_src = xa_d.ap().rearrange("p (r c) -> p r c", c=XCOLS)
            xb_src = xb_d.ap().rearrange("p (r c) -> p r c", c=XCOLS)
            xc_src = xc_d.ap().rearrange("p (r c) -> p r c", c=XCOLS)
            CH0 = 17
            nc.sync.dma_start(out=xa[:, 0:CH0, :], in_=xa_src[:, 0:CH0, :])
            nc.scalar.dma_start(
                out=wl, in_=wl_d.ap().rearrange("p (j m) -> p j m", m=128))
            nc.sync.dma_start(out=xb[:, 0:CH0, :], in_=xb_src[:, 0:CH0, :])
            nc.sync.dma_start(out=xc[:, 0:CH0, :], in_=xc_src[:, 0:CH0, :])
            for c0 in range(CH0, XROWS, 17):
                ce = min(c0 + 17, XROWS)
                nc.sync.dma_start(
                    out=xa[:, c0:ce, :], in_=xa_src[:, c0:ce, :])
                nc.sync.dma_start(
                    out=xb[:, c0:ce, :], in_=xb_src[:, c0:ce, :])
                nc.sync.dma_start(
                    out=xc[:, c0:ce, :], in_=xc_src[:, c0:ce, :])
            bd = consts.tile([128, 8], bf16)
            nc.sync.dma_start(out=bd, in_=bd_d.ap())
            # expand matrix replicated at base partitions 0/32/64 (matmul
            # requires lhsT and rhs to share a base partition)
            ex = consts.tile([SLOT * (GRP - 1) + 8, 128], bf16)
            for j in range(GRP):
                nc.sync.dma_start(out=ex[SLOT * j:SLOT * j + 8, :],
                                  in_=ex_d.ap())
            s2x = consts.tile([128, N_GRPS, BLK * 128], f32)
            nc.sync.dma_start(
                out=s2x,
                in_=s2x_d.ap().rearrange("p (g e) -> p g e", e=BLK * 128))
            eps_t = consts.tile([128, 1], f32)
            nc.vector.memset(eps_t[:], 1e-9)

            out_v = out_d.ap().rearrange("p (r c) -> p r c", c=128)

            # PE warm-up: ~40 throwaway matmuls on scratch SBUF while the
            # input DMA lands. Keeps the PE continuously busy from ~+1us so
            # the HAM throttle reaches its full-clock state before the first
            # real conv matmul (cold-start matmuls otherwise run ~1.5x slow).
            scratch = consts.tile([128, 512], bf16)
            nc.vector.memset(scratch[:], 0.0)
            warm_ps = pf.tile([128, BLK, 128], f32, name="warm_ps", tag="fe")
            for _ in range(10):
                nc.tensor.matmul(warm_ps[:], scratch[:, 0:128], scratch[:],
                                 start=True, stop=True)

            y_all = [None] * N_GRPS
            F_g = [None] * N_GRPS
            p_sb = [None] * N_BLKS

            def stageA(blk):
                """conv 13 MMs -> p_ps; psq (ACT); p_sb copy (DVE); bd MM into
                the group's shared y PSUM tile at partitions [8j, 8j+8)."""
                g, j = blk // GRP, blk % GRP
                r0 = blk * BLK
                p_ps = pp.tile([128, BLK, 128], f32)
                for m, (src, roff, coff) in enumerate(_MM_SLICES):
                    xsrc = {'XA': xa, 'XB': xb, 'XC': xc}[src]
                    if m == 12:  # K=64 single on lo partitions
                        lhsT = wl[0:64, m, :]
                        rhs = xsrc[0:64, r0 + roff:r0 + roff + BLK,
                                   coff:coff + 128]
                    else:
                        lhsT = wl[:, m, :]
                        rhs = xsrc[:, r0 + roff:r0 + roff + BLK, coff:coff + 128]
                    nc.tensor.matmul(p_ps[:], lhsT, rhs,
                                     start=(m == 0), stop=(m == 12))
                psq = work.tile([128, BLK, 128], bf16, tag="psq")
                nc.scalar.activation(psq[:], p_ps[:], AF.Square)
                pb = pall.tile([128, BLK, 128], bf16, tag="p_sb")
                nc.scalar.activation(pb[:], p_ps[:], AF.Copy, bias=0.0)
                p_sb[blk] = pb
                if j == 0:
                    y_all[g] = py.tile([128, BLK, 128], f32, tag="y_all",
                                       name=f"y_all_{g}")
                nc.tensor.matmul(y_all[g][SLOT * j:SLOT * j + 8, :, :], bd[:],
                                 psq[:], start=True, stop=True,
                                 tile_position=(0, SLOT * j))

            def stageB(g):
                """factor on all 128 partitions: F = y/((1+y)*sqrt(y_raw+eps)),
                y = s^2 * y_raw, for 8 blocks at once."""
                y_ps = y_all[g]
                a_t = fac.tile([128, BLK, 128], f32, tag="a")
                nc.scalar.activation(a_t[:], y_ps[:], AF.Sqrt, bias=eps_t[:])
                y_t = fac.tile([128, BLK, 128], f32, tag="y")
                nc.vector.tensor_mul(
                    y_t[:], y_ps[:],
                    s2x[:, g, :].rearrange("p (r c) -> p r c", c=128))
                y1_t = fac.tile([128, BLK, 128], f32, tag="y1")
                nc.vector.tensor_scalar_add(y1_t[:], y_t[:], 1.0)
                b_t = fac.tile([128, BLK, 128], f32, tag="b")
                nc.vector.tensor_mul(b_t[:], a_t[:], y1_t[:])
                r_t = fac.tile([128, BLK, 128], f32, tag="r")
                nc.vector.reciprocal_approx_fast(r_t[:], b_t[:])
                Ft = fac.tile([128, BLK, 128], bf16, tag="F")
                nc.vector.tensor_mul(Ft[:], y_t[:], r_t[:])
                F_g[g] = Ft

            def stageC(blk):
                """expand F (K=8 MM) and v = p * F -> bf16 -> HBM."""
                g, j = blk // GRP, blk % GRP
                r0 = blk * BLK
                fe_ps = pf.tile([128, BLK, 128], f32, tag="fe")
                nc.tensor.matmul(fe_ps[:], ex[SLOT * j:SLOT * j + 8, :],
                                 F_g[g][SLOT * j:SLOT * j + 8, :, :],
                                 start=True, stop=True,
                                 tile_position=(SLOT * j, 0))
                v_t = work.tile([128, BLK, 128], bf16, tag="v")
                nc.vector.tensor_mul(v_t[:], p_sb[blk][:], fe_ps[:])
                nc.sync.dma_start(out=out_v[:, r0:r0 + BLK, :], in_=v_t[:])

            for blk in range(N_BLKS):
                stageA(blk)
                if blk % GRP == GRP - 1 or blk == N_BLKS - 1:
                    stageB(blk // GRP)
                if blk >= GRP:
                    stageC(blk - GRP)
            for blk in range(N_BLKS - GRP, N_BLKS):
                stageC(blk)

    nc.compile()
    return nc


def _prep_in_maps(u, W):
    x = u.reshape(N, CIN, H, W_SP)
    wl = _weight_tiles(W).reshape(128, 13 * 128).astype(BF16)
    bd = _block_diag().astype(BF16)
    ex = np.ascontiguousarray(_block_diag().T).astype(BF16)
    in_maps = []
    for core in range(N_CORES):
        n, half = core // 2, core % 2
        XA, XB, XC = _inputs_core(x[n], half)
        in_maps.append({
            "xa": XA.reshape(128, XROWS * XCOLS).astype(BF16),
            "xb": XB.reshape(128, XROWS * XCOLS).astype(BF16),
            "xc": XC.reshape(128, XROWS * XCOLS).astype(BF16),
            "wl": wl,
            "bd": bd,
            "ex": ex,
            "s2x": _s2x_map(half),
        })
    return in_maps


def run(u, W, trace=False):
    """Returns (out [N,T1,Z1,H,W] f32, BassKernelResults)."""
    from concourse.bass_utils import run_bass_kernel_spmd

    if "nc" not in _CACHE:
        _CACHE["nc"] = build_nc()
    nc = _CACHE["nc"]
    in_maps = _prep_in_maps(np.asarray(u, np.float32), np.asarray(W, np.float32))
    res = run_bass_kernel_spmd(nc, in_maps, list(range(N_CORES)), trace=trace)
    out = np.empty((N, T1, Z1, H, W_SP), np.float32)
    for core in range(N_CORES):
        n, half = core // 2, core % 2
        o = np.asarray(res.results[core]["out"]).astype(np.float32)
        o = o.reshape(T1, Z1, ROWS, 128)
        out[n, :, :, half * 64:(half + 1) * 64, :] = o
    return out, res


def kernel(u, W):
    out, _ = run(u, W, trace=False)
    return out
